# revision 1
# baseline (speedup 1.0000x reference)
"""Trainium2 Bass kernel for fused CrossEntropy + CRL + MDCA loss.

Strategy (data-parallel over 8 NeuronCores):
  - logits [4096, 32000] f32 sharded by batch: 512 rows/core.
  - Per core (launch A):
      * stream logits in [128 x 2000] chunks (DMA ~65.5MB/core, the roofline)
      * ACT: e = exp(x) in bf16 (kept in SBUF), fused accum_out -> row sums s
      * DVE: running row max over chunks -> mx; r = 1/s; conf = exp(mx)*r
      * PE : per-class column sums of p = e*r via 1000 small matmuls
             (lhsT = e[128 rows, 128 classes], rhs = r[128,1]; per-tile
             ping-pong PSUM, DVE folds tiles into an SBUF accumulator)
      * GPSIMD: indirect-DMA gathers of logits[i, target_i] and
        correctness[idx_i] (one offset per partition per transfer)
      * outputs: colsum[128,250], conf[128,4], sum(lse - x_target) [1,1],
        correctness[idx] [128,4]

  Hardware sync notes (cost a day of debugging, do not regress):
    - dependent ops on the SAME engine need a semaphore self-handshake
      (inc on producer, wait before consumer): engine pipelines can read
      SBUF before the previous op's write retires.
    - input ring slots each use their OWN semaphore: with one cumulative
      DMA sem, later chunks' per-SDMA-engine increments can satisfy an
      earlier chunk's wait while that chunk is still in flight.
    - indirect DMA gathers exactly one offset per partition (the free dim
      reads consecutive elements); multi-offset-per-partition silently
      reads consecutive data on HW even though CoreSim honors it.
  - Launch B (1 core): cross-core reduction of colsums, MDCA |avg_conf -
    counts/B| term, CRL margin-ranking term from the full conf vector +
    correctness-table gathers, final scalar combine.
  - Host work is limited to sharding, index/offset prep (np.bincount of the
    int targets, flat gather offsets, np.roll for the rank pairing) and
    concatenating per-core outputs between the two launches.
"""

import numpy as np

import concourse.bass as bass
from concourse import mybir
from concourse.bass_utils import run_bass_kernel_spmd

# Problem constants (hardcoded per contract).
B, C = 4096, 32000
DATASET = 50000
N_CORES = 8
R = B // N_CORES          # 512 rows per core
P = 128                   # partitions
T = R // P                # 4 row tiles per core
CW = 2000                 # column chunk width
NW = C // CW              # 8 chunks per row tile
NCH = T * NW              # 32 chunks per core
NB = 6                    # input ring buffers
CB = C // P               # 250 class blocks

FP32 = mybir.dt.float32
BF16 = mybir.dt.bfloat16
INT32 = mybir.dt.int32


def _build_launch_a(detect_races: bool = True,
                    debug_outs: bool = False) -> bass.Bass:
    from contextlib import ExitStack

    nc = bass.Bass("TRN2", target_bir_lowering=False, debug=False,
                   num_devices=N_CORES,
                   detect_race_conditions=detect_races)
    xl = nc.dram_tensor("xl", [R, C], FP32, kind="ExternalInput")
    xti = nc.dram_tensor("xti", [P, T], INT32, kind="ExternalInput")
    idxo = nc.dram_tensor("idxo", [P, T], INT32, kind="ExternalInput")
    hist = nc.dram_tensor("hist", [DATASET], FP32, kind="ExternalInput")
    out_colsum = nc.dram_tensor("out_colsum", [P, CB], FP32,
                                kind="ExternalOutput")
    out_conf = nc.dram_tensor("out_conf", [P, T], FP32, kind="ExternalOutput")
    out_cls = nc.dram_tensor("out_cls", [1, 1], FP32, kind="ExternalOutput")
    out_c1 = nc.dram_tensor("out_c1", [P, T], FP32, kind="ExternalOutput")
    if debug_outs:
        dbg_sacc = nc.dram_tensor("dbg_sacc", [P, NCH], FP32,
                                  kind="ExternalOutput")
        dbg_st = nc.dram_tensor("dbg_st", [P, T], FP32, kind="ExternalOutput")
        dbg_mx = nc.dram_tensor("dbg_mx", [P, T], FP32, kind="ExternalOutput")
        dbg_lse = nc.dram_tensor("dbg_lse", [P, T], FP32,
                                 kind="ExternalOutput")
        dbg_e0 = nc.dram_tensor("dbg_e0", [P, C], BF16, kind="ExternalOutput")
        dbg_xt = nc.dram_tensor("dbg_xt", [P, T], FP32, kind="ExternalOutput")
        dbg_expmx = nc.dram_tensor("dbg_expmx", [P, T], FP32,
                                   kind="ExternalOutput")
        dbg_rt = nc.dram_tensor("dbg_rt", [P, T], FP32, kind="ExternalOutput")
        dbg_d4 = nc.dram_tensor("dbg_d4", [P, T], FP32, kind="ExternalOutput")
        dbg_d1 = nc.dram_tensor("dbg_d1", [P, 1], FP32, kind="ExternalOutput")

    xl_flat = xl.ap().rearrange("a (b c) -> (a b) c", c=1)
    hist_flat = hist.ap().rearrange("(a b) -> a b", b=1)

    with ExitStack() as ctx:
        xbuf = ctx.enter_context(nc.sbuf_tensor([P, NB * CW], FP32))
        e0 = ctx.enter_context(nc.sbuf_tensor([P, C], BF16))
        e1 = ctx.enter_context(nc.sbuf_tensor([P, C], BF16))
        ebufs = [e0, e1]
        sacc = ctx.enter_context(nc.sbuf_tensor([P, NCH], FP32))
        mxp = ctx.enter_context(nc.sbuf_tensor([P, NCH], FP32))
        s_t = ctx.enter_context(nc.sbuf_tensor([P, T], FP32))
        mx_t = ctx.enter_context(nc.sbuf_tensor([P, T], FP32))
        r_t = ctx.enter_context(nc.sbuf_tensor([P, T], FP32))
        rb_t = ctx.enter_context(nc.sbuf_tensor([P, T], BF16))
        expmx = ctx.enter_context(nc.sbuf_tensor([P, T], FP32))
        lse_t = ctx.enter_context(nc.sbuf_tensor([P, T], FP32))
        conf_t = ctx.enter_context(nc.sbuf_tensor([P, T], FP32))
        xt_g = ctx.enter_context(nc.sbuf_tensor([P, T], FP32))
        xti_s = ctx.enter_context(nc.sbuf_tensor([P, T], INT32))
        idxo_s = ctx.enter_context(nc.sbuf_tensor([P, T], INT32))
        c1p = ctx.enter_context(nc.sbuf_tensor([P, T], FP32))
        csum_sb = ctx.enter_context(nc.sbuf_tensor([P, CB], FP32))
        d4 = ctx.enter_context(nc.sbuf_tensor([P, T], FP32))
        d1 = ctx.enter_context(nc.sbuf_tensor([P, 1], FP32))
        ones_sb = ctx.enter_context(nc.sbuf_tensor([P, 1], FP32))
        cls_sb = ctx.enter_context(nc.sbuf_tensor([1, 1], FP32))
        psum0 = ctx.enter_context(nc.psum_tensor([P, CB], FP32))
        psum1 = ctx.enter_context(nc.psum_tensor([P, CB], FP32))
        psum_cls = ctx.enter_context(nc.psum_tensor([1, 1], FP32))

        sems_in = [ctx.enter_context(nc.semaphore(f"sem_in{i}"))
                   for i in range(NB)]
        sem_misc = ctx.enter_context(nc.semaphore("sem_misc"))
        sem_act = ctx.enter_context(nc.semaphore("sem_act"))
        sem_dvec = ctx.enter_context(nc.semaphore("sem_dvec"))
        sem_dves = ctx.enter_context(nc.semaphore("sem_dves"))
        sem_acts = ctx.enter_context(nc.semaphore("sem_acts"))
        sem_pe = ctx.enter_context(nc.semaphore("sem_pe"))
        sem_gp = ctx.enter_context(nc.semaphore("sem_gp"))
        sem_csum = ctx.enter_context(nc.semaphore("sem_csum"))
        sem_conf = ctx.enter_context(nc.semaphore("sem_conf"))
        sem_d1 = ctx.enter_context(nc.semaphore("sem_d1"))
        sem_cls = ctx.enter_context(nc.semaphore("sem_cls"))
        sem_od = ctx.enter_context(nc.semaphore("sem_od"))
        sem_pecls = ctx.enter_context(nc.semaphore("sem_pecls"))
        sem_dveacc = ctx.enter_context(nc.semaphore("sem_dveacc"))
        sem_dvs = ctx.enter_context(nc.semaphore("sem_dvs"))

        block = ctx.enter_context(nc.Block())

        @block.sync
        def _(sync):
            for k in range(NCH):
                t, w = divmod(k, NW)
                if k == NB:
                    # off the critical start: index DMAs only feed the
                    # gpsimd gathers, which are consumed near the end
                    sync.dma_start(xti_s[:], xti[:]).then_inc(sem_misc, 16)
                    sync.dma_start(idxo_s[:], idxo[:]).then_inc(sem_misc, 16)
                if k >= NB:
                    sync.wait_ge(sem_act, k - NB + 1)
                    sync.wait_ge(sem_dvec, k - NB + 1)
                b = k % NB
                sync.dma_start(
                    xbuf[:, b * CW:(b + 1) * CW],
                    xl[t * P:(t + 1) * P, w * CW:(w + 1) * CW],
                ).then_inc(sems_in[b], 16)
            sync.wait_ge(sem_gp, 16 * 2 * T)
            sync.dma_start(out_c1[:], c1p[:]).then_inc(sem_od, 16)
            sync.wait_ge(sem_conf, 1)
            sync.dma_start(out_conf[:], conf_t[:]).then_inc(sem_od, 16)
            sync.wait_ge(sem_cls, 1)
            sync.dma_start(out_cls[:], cls_sb[:]).then_inc(sem_od, 16)
            sync.wait_ge(sem_csum, 1)
            sync.dma_start(out_colsum[:], csum_sb[:]).then_inc(sem_od, 16)
            if debug_outs:
                sync.dma_start(dbg_sacc[:], sacc[:]).then_inc(sem_od, 16)
                sync.dma_start(dbg_st[:], s_t[:]).then_inc(sem_od, 16)
                sync.dma_start(dbg_mx[:], mx_t[:]).then_inc(sem_od, 16)
                sync.dma_start(dbg_lse[:], lse_t[:]).then_inc(sem_od, 16)
                sync.dma_start(dbg_e0[:], ebufs[1][:]).then_inc(sem_od, 16)
                sync.dma_start(dbg_xt[:], xt_g[:]).then_inc(sem_od, 16)
                sync.dma_start(dbg_expmx[:], expmx[:]).then_inc(sem_od, 16)
                sync.dma_start(dbg_rt[:], r_t[:]).then_inc(sem_od, 16)
                sync.dma_start(dbg_d4[:], d4[:]).then_inc(sem_od, 16)
                sync.dma_start(dbg_d1[:], d1[:]).then_inc(sem_od, 16)

        @block.scalar
        def _(scalar):
            for k in range(NCH):
                t, w = divmod(k, NW)
                if w == 0 and t >= 2:
                    # e[t%2] still being read by PE for tile t-2
                    scalar.wait_ge(sem_pe, t - 1)
                b = k % NB
                scalar.wait_ge(sems_in[b], 16 * (k // NB + 1))
                scalar.activation(
                    out=ebufs[t % 2][:, w * CW:(w + 1) * CW],
                    in_=xbuf[:, b * CW:(b + 1) * CW],
                    func=mybir.ActivationFunctionType.Exp,
                    accum_out=sacc[:, k:k + 1],
                ).then_inc(sem_act, 1)
            # per-tile stats: exp(mx) and ln(s) after DVE computed mx, s
            for t in range(T):
                scalar.wait_ge(sem_dves, t + 1)
                scalar.activation(
                    out=expmx[:, t:t + 1], in_=mx_t[:, t:t + 1],
                    func=mybir.ActivationFunctionType.Exp,
                )
                scalar.activation(
                    out=lse_t[:, t:t + 1], in_=s_t[:, t:t + 1],
                    func=mybir.ActivationFunctionType.Ln,
                ).then_inc(sem_acts, 1)

        @block.vector
        def _(vector):
            vector.memset(ones_sb[:], 1.0)
            for t in range(T):
                for w in range(NW):
                    k = t * NW + w
                    b = k % NB
                    vector.wait_ge(sems_in[b], 16 * (k // NB + 1))
                    vector.tensor_reduce(
                        out=mxp[:, k:k + 1],
                        in_=xbuf[:, b * CW:(b + 1) * CW],
                        axis=mybir.AxisListType.X,
                        op=mybir.AluOpType.max,
                    ).then_inc(sem_dvec, 1)
                # tile stats (needs ACT's sacc for this tile)
                vector.wait_ge(sem_act, NW * (t + 1))
                # self-sync: own chunk-max writes to mxp must be committed
                vector.wait_ge(sem_dvec, NW * (t + 1))
                vector.tensor_reduce(
                    out=mx_t[:, t:t + 1], in_=mxp[:, t * NW:(t + 1) * NW],
                    axis=mybir.AxisListType.X, op=mybir.AluOpType.max,
                )
                vector.tensor_reduce(
                    out=s_t[:, t:t + 1], in_=sacc[:, t * NW:(t + 1) * NW],
                    axis=mybir.AxisListType.X, op=mybir.AluOpType.add,
                ).then_inc(sem_dvs, 1)
                vector.wait_ge(sem_dvs, 2 * t + 1)
                vector.reciprocal(
                    out=r_t[:, t:t + 1], in_=s_t[:, t:t + 1]
                ).then_inc(sem_dvs, 1)
                vector.wait_ge(sem_dvs, 2 * t + 2)
                vector.tensor_copy(
                    out=rb_t[:, t:t + 1], in_=r_t[:, t:t + 1]
                ).then_inc(sem_dves, 1)
                if t >= 1:
                    # fold tile t-1's per-class sums into the accumulator
                    vector.wait_ge(sem_pe, t)
                    psrc = [psum0, psum1][(t - 1) % 2]
                    if t == 1:
                        acc_inst = vector.tensor_copy(
                            out=csum_sb[:], in_=psrc[:])
                    else:
                        acc_inst = vector.tensor_tensor(
                            out=csum_sb[:], in0=csum_sb[:], in1=psrc[:],
                            op=mybir.AluOpType.add,
                        )
                    acc_inst.then_inc(sem_dveacc, 1)
            # conf = exp(mx) * r, all tiles at once
            vector.wait_ge(sem_acts, T)
            vector.tensor_tensor(
                out=conf_t[:], in0=expmx[:], in1=r_t[:],
                op=mybir.AluOpType.mult,
            ).then_inc(sem_conf, 1)
            # cls partial: sum over rows of (lse - x_target)
            vector.wait_ge(sem_gp, 16 * T)
            vector.tensor_tensor(
                out=d4[:], in0=lse_t[:], in1=xt_g[:],
                op=mybir.AluOpType.subtract,
            ).then_inc(sem_dvs, 1)
            vector.wait_ge(sem_dvs, 2 * T + 1)
            vector.tensor_reduce(
                out=d1[:], in_=d4[:], axis=mybir.AxisListType.X,
                op=mybir.AluOpType.add,
            ).then_inc(sem_d1, 1)
            # fold final tile's per-class sums
            vector.wait_ge(sem_pe, T)
            vector.tensor_tensor(
                out=csum_sb[:], in0=csum_sb[:], in1=[psum0, psum1][(T - 1) % 2][:],
                op=mybir.AluOpType.add,
            ).then_inc(sem_csum, 1)
            vector.wait_ge(sem_pecls, 1)
            vector.tensor_copy(out=cls_sb[:], in_=psum_cls[0:1, 0:1]).then_inc(
                sem_cls, 1)

        @block.tensor
        def _(tensor):
            psums = [psum0, psum1]
            for t in range(T):
                tensor.wait_ge(sem_act, NW * (t + 1))
                tensor.wait_ge(sem_dves, t + 1)
                if t >= 2:
                    tensor.wait_ge(sem_dveacc, t - 1)
                eb = ebufs[t % 2]
                pt = psums[t % 2]
                for c in range(CB):
                    inst = tensor.matmul(
                        out=pt[:, c:c + 1],
                        lhsT=eb[:, c * P:(c + 1) * P],
                        rhs=rb_t[:, t:t + 1],
                        start=True,
                        stop=True,
                    )
                inst.then_inc(sem_pe, 1)
            # cross-partition sum of d1 via ones-matmul
            tensor.wait_ge(sem_d1, 1)
            tensor.matmul(
                out=psum_cls[0:1, 0:1],
                lhsT=ones_sb[:, 0:1],
                rhs=d1[:, 0:1],
                start=True,
                stop=True,
            ).then_inc(sem_pecls, 1)

        @block.gpsimd
        def _(gpsimd):
            gpsimd.wait_ge(sem_misc, 32)
            for t in range(T):
                gpsimd.indirect_dma_start(
                    out=xt_g[:, t:t + 1],
                    out_offset=None,
                    in_=xl_flat,
                    in_offset=bass.IndirectOffsetOnAxis(
                        ap=xti_s[:, t:t + 1], axis=0),
                ).then_inc(sem_gp, 16)
            for t in range(T):
                gpsimd.indirect_dma_start(
                    out=c1p[:, t:t + 1],
                    out_offset=None,
                    in_=hist_flat,
                    in_offset=bass.IndirectOffsetOnAxis(
                        ap=idxo_s[:, t:t + 1], axis=0),
                ).then_inc(sem_gp, 16)

    return nc


def _build_launch_b(detect_races: bool = True,
                    debug_outs: bool = False) -> bass.Bass:
    from contextlib import ExitStack

    NF = B // P  # 32 pairs per partition

    nc = bass.Bass("TRN2", target_bir_lowering=False, debug=False,
                   num_devices=1,
                   detect_race_conditions=detect_races)
    csums = nc.dram_tensor("csums", [P, CB * N_CORES], FP32,
                           kind="ExternalInput")
    cnts = nc.dram_tensor("cnts", [P, CB], FP32, kind="ExternalInput")
    cfa = nc.dram_tensor("cfa", [P, NF], FP32, kind="ExternalInput")
    cfb = nc.dram_tensor("cfb", [P, NF], FP32, kind="ExternalInput")
    c1v = nc.dram_tensor("c1v", [P, NF], FP32, kind="ExternalInput")
    c2v = nc.dram_tensor("c2v", [P, NF], FP32, kind="ExternalInput")
    clsp = nc.dram_tensor("clsp", [1, N_CORES], FP32, kind="ExternalInput")
    out_loss = nc.dram_tensor("out_loss", [1, 1], FP32, kind="ExternalOutput")
    if debug_outs:
        dbg_c1 = nc.dram_tensor("dbg_c1", [P, NF], FP32, kind="ExternalOutput")
        dbg_tred = nc.dram_tensor("dbg_tred", [P, CB], FP32,
                                  kind="ExternalOutput")
        dbg_terms = nc.dram_tensor("dbg_terms", [P, NF], FP32,
                                   kind="ExternalOutput")
        dbg_u = nc.dram_tensor("dbg_u", [P, 1], FP32, kind="ExternalOutput")
        dbg_rcls = nc.dram_tensor("dbg_rcls", [1, 1], FP32,
                                  kind="ExternalOutput")

    with ExitStack() as ctx:
        sb_csums = ctx.enter_context(nc.sbuf_tensor([P, CB * N_CORES], FP32))
        sb_cnts = ctx.enter_context(nc.sbuf_tensor([P, CB], FP32))
        sb_cfa = ctx.enter_context(nc.sbuf_tensor([P, NF], FP32))
        sb_cfb = ctx.enter_context(nc.sbuf_tensor([P, NF], FP32))
        sb_clsp = ctx.enter_context(nc.sbuf_tensor([1, N_CORES], FP32))
        c1 = ctx.enter_context(nc.sbuf_tensor([P, NF], FP32))
        c2 = ctx.enter_context(nc.sbuf_tensor([P, NF], FP32))
        t_red = ctx.enter_context(nc.sbuf_tensor([P, CB], FP32))
        t_abs = ctx.enter_context(nc.sbuf_tensor([P, CB], FP32))
        t_neg = ctx.enter_context(nc.sbuf_tensor([P, CB], FP32))
        s12 = ctx.enter_context(nc.sbuf_tensor([P, NF], FP32))
        gtp = ctx.enter_context(nc.sbuf_tensor([P, NF], FP32))
        ltp = ctx.enter_context(nc.sbuf_tensor([P, NF], FP32))
        sgn = ctx.enter_context(nc.sbuf_tensor([P, NF], FP32))
        dd = ctx.enter_context(nc.sbuf_tensor([P, NF], FP32))
        prod = ctx.enter_context(nc.sbuf_tensor([P, NF], FP32))
        terms = ctx.enter_context(nc.sbuf_tensor([P, NF], FP32))
        r_cal = ctx.enter_context(nc.sbuf_tensor([P, 1], FP32))
        r_ref = ctx.enter_context(nc.sbuf_tensor([P, 1], FP32))
        u = ctx.enter_context(nc.sbuf_tensor([P, 1], FP32))
        r_cls = ctx.enter_context(nc.sbuf_tensor([1, 1], FP32))
        sc = ctx.enter_context(nc.sbuf_tensor([1, 1], FP32))
        ones_sb = ctx.enter_context(nc.sbuf_tensor([P, 1], FP32))
        psum_b = ctx.enter_context(nc.psum_tensor([1, 1], FP32))

        sem_in = ctx.enter_context(nc.semaphore("sem_in"))
        sem_v = ctx.enter_context(nc.semaphore("sem_v"))
        sem_c = ctx.enter_context(nc.semaphore("sem_c"))
        sem_in2 = ctx.enter_context(nc.semaphore("sem_in2"))
        sem_p = ctx.enter_context(nc.semaphore("sem_p"))
        sem_od = ctx.enter_context(nc.semaphore("sem_od"))

        block = ctx.enter_context(nc.Block())

        @block.sync
        def _(sync):
            sync.dma_start(sb_cnts[:], cnts[:]).then_inc(sem_in, 16)
            sync.dma_start(sb_cfa[:], cfa[:]).then_inc(sem_in, 16)
            sync.dma_start(sb_cfb[:], cfb[:]).then_inc(sem_in, 16)
            sync.dma_start(c1[:], c1v[:]).then_inc(sem_in, 16)
            sync.dma_start(c2[:], c2v[:]).then_inc(sem_in, 16)
            sync.dma_start(sb_clsp[:], clsp[:]).then_inc(sem_in, 16)
            sync.dma_start(sb_csums[:], csums[:]).then_inc(sem_in2, 16)
            sync.wait_ge(sem_v, 5)
            sync.dma_start(out_loss[:], sc[:]).then_inc(sem_od, 16)
            if debug_outs:
                sync.dma_start(dbg_c1[:], c1[:]).then_inc(sem_od, 16)
                sync.dma_start(dbg_tred[:], t_red[:]).then_inc(sem_od, 16)
                sync.dma_start(dbg_terms[:], terms[:]).then_inc(sem_od, 16)
                sync.dma_start(dbg_u[:], u[:]).then_inc(sem_od, 16)
                sync.dma_start(dbg_rcls[:], r_cls[:]).then_inc(sem_od, 16)

        @block.tensor
        def _(tensor):
            tensor.wait_ge(sem_v, 4)
            tensor.matmul(
                out=psum_b[0:1, 0:1],
                lhsT=ones_sb[:, 0:1],
                rhs=u[:, 0:1],
                start=True,
                stop=True,
            ).then_inc(sem_p, 1)

        @block.vector
        def _(vector):
            vector.memset(ones_sb[:], 1.0)
            vector.wait_ge(sem_in, 96)
            n = [0]

            def step(inst):
                n[0] += 1
                inst.then_inc(sem_c, 1)
                vector.wait_ge(sem_c, n[0])

            # CRL first: needs only the six small inputs
            step(vector.tensor_tensor(out=s12[:], in0=c1[:], in1=c2[:],
                                      op=mybir.AluOpType.subtract))
            step(vector.tensor_scalar(out=gtp[:], in0=s12[:], scalar1=0.0,
                                      scalar2=None,
                                      op0=mybir.AluOpType.is_gt))
            step(vector.tensor_scalar(out=ltp[:], in0=s12[:], scalar1=0.0,
                                      scalar2=None,
                                      op0=mybir.AluOpType.is_lt))
            step(vector.tensor_tensor(out=sgn[:], in0=gtp[:], in1=ltp[:],
                                      op=mybir.AluOpType.subtract))
            step(vector.tensor_tensor(out=dd[:], in0=sb_cfa[:],
                                      in1=sb_cfb[:],
                                      op=mybir.AluOpType.subtract))
            step(vector.tensor_tensor(out=dd[:], in0=dd[:], in1=s12[:],
                                      op=mybir.AluOpType.subtract))
            step(vector.tensor_tensor(out=prod[:], in0=sgn[:], in1=dd[:],
                                      op=mybir.AluOpType.mult))
            # relu(-prod) = max(-prod, 0)
            step(vector.tensor_scalar(out=terms[:], in0=prod[:],
                                      scalar1=-1.0, scalar2=0.0,
                                      op0=mybir.AluOpType.mult,
                                      op1=mybir.AluOpType.max))
            step(vector.tensor_reduce(
                out=r_ref[:], in_=terms[:], axis=mybir.AxisListType.X,
                op=mybir.AluOpType.add,
            ))
            # MDCA: avg_conf over cores, then sum |avg_conf - counts| fused
            vector.wait_ge(sem_in2, 16)
            step(vector.tensor_reduce(
                out=t_red[:],
                in_=sb_csums.ap().rearrange("p (a b) -> p a b", b=N_CORES),
                axis=mybir.AxisListType.X, op=mybir.AluOpType.add,
            ))
            step(vector.tensor_tensor(
                out=t_abs[:], in0=t_red[:], in1=sb_cnts[:],
                op=mybir.AluOpType.subtract,
            ))
            step(vector.tensor_reduce(
                out=r_cal[:], in_=t_abs[:], axis=mybir.AxisListType.X,
                op=mybir.AluOpType.add, apply_absolute_value=True,
            ))
            # u = r_cal/(C*B) + r_ref/B
            step(vector.tensor_scalar_mul(u[:], r_cal[:], 1.0 / (C * B)))
            vector.scalar_tensor_tensor(
                out=u[:], in0=r_ref[:], scalar=1.0 / B, in1=u[:],
                op0=mybir.AluOpType.mult, op1=mybir.AluOpType.add,
            ).then_inc(sem_v, 4)
            # final: sc = psum_b[0] + sum(clsp)/B
            vector.wait_ge(sem_p, 1)
            step(vector.tensor_reduce(
                out=r_cls[:], in_=sb_clsp[:], axis=mybir.AxisListType.X,
                op=mybir.AluOpType.add,
            ))
            vector.scalar_tensor_tensor(
                out=sc[:], in0=r_cls[:], scalar=1.0 / B, in1=psum_b[0:1, :],
                op0=mybir.AluOpType.mult, op1=mybir.AluOpType.add,
            ).then_inc(sem_v, 1)

    return nc


_CACHE: dict[str, bass.Bass] = {}


def _get(name, builder):
    if name not in _CACHE:
        _CACHE[name] = builder()
    return _CACHE[name]


def kernel(logits, targets, idx, correctness):
    logits = np.ascontiguousarray(np.asarray(logits, dtype=np.float32))
    targets = np.asarray(targets).astype(np.int64)
    idx = np.asarray(idx).astype(np.int64)
    correctness = np.asarray(correctness, dtype=np.float32)

    nc_a = _get("a", _build_launch_a)
    nc_b = _get("b", _build_launch_b)

    # ---- launch A: 8-core data-parallel heavy pass --------------------
    rows = np.arange(R, dtype=np.int64)
    in_maps = []
    for k in range(N_CORES):
        sl = slice(k * R, (k + 1) * R)
        tg = targets[sl]
        off = rows * C + tg  # flat element offsets into this core's shard
        xti = off.reshape(T, P).T.astype(np.int32)  # [P, T], row = t*128+p
        xti = np.ascontiguousarray(xti)
        idxo = np.ascontiguousarray(
            idx[sl].reshape(T, P).T.astype(np.int32))
        in_maps.append({"xl": logits[sl], "xti": xti, "idxo": idxo,
                        "hist": correctness})

    res_a = run_bass_kernel_spmd(nc_a, in_maps, list(range(N_CORES)))

    colsums = np.stack(
        [res_a.results[k]["out_colsum"] for k in range(N_CORES)], axis=-1
    )  # [P, CB, N_CORES]
    conf_global = np.concatenate(
        [res_a.results[k]["out_conf"].T.reshape(R) for k in range(N_CORES)]
    )  # [B] in global row order
    clsp = np.array(
        [[res_a.results[k]["out_cls"][0, 0] for k in range(N_CORES)]],
        dtype=np.float32,
    )
    c1_global = np.concatenate(
        [res_a.results[k]["out_c1"].T.reshape(R) for k in range(N_CORES)]
    )  # correctness[idx] in global row order

    # ---- host glue: histogram of int targets, rank-pair rolls ---------
    counts = np.bincount(targets, minlength=C).astype(np.float32)
    cnts = np.ascontiguousarray(counts.reshape(CB, P).T)  # [P, CB]

    conf2_global = np.roll(conf_global, -1)
    c2_global = np.roll(c1_global, -1)
    NF = B // P
    cfa = np.ascontiguousarray(conf_global.reshape(P, NF).astype(np.float32))
    cfb = np.ascontiguousarray(conf2_global.reshape(P, NF).astype(np.float32))
    c1v = np.ascontiguousarray(c1_global.reshape(P, NF).astype(np.float32))
    c2v = np.ascontiguousarray(c2_global.reshape(P, NF).astype(np.float32))

    in_b = {
        "csums": np.ascontiguousarray(
            colsums.reshape(P, CB * N_CORES).astype(np.float32)),
        "cnts": cnts,
        "cfa": cfa,
        "cfb": cfb,
        "c1v": c1v,
        "c2v": c2v,
        "clsp": clsp,
    }
    res_b = run_bass_kernel_spmd(nc_b, [in_b], [0])
    total = res_b.results[0]["out_loss"][0, 0]
    return np.array(total, dtype=np.float32)

